# revision 47
# baseline (speedup 1.0000x reference)
"""Single-head causal self-attention on 8 Trainium2 NeuronCores (Bass/Tile).

Problem: x [1024, 256, 384], Wq/Wk/Wv [384, 64] ->
  q,k,v = x@W;  wei = softmax(mask(q k^T / sqrt(384)));  out = wei @ v
Output: [1024, 256, 64] fp32.

Design (4.3x over the fp32r v1 baseline: 661us -> ~155us):
  - Data-parallel over batch: 128 batches per core, processed as 32
    groups of 4 with a 4-deep software pipeline (DMA -> projections ->
    attention -> output, P matrices consumed two iterations after they
    are produced so the softmax chain never stalls the PE).
  - fp16 operands everywhere (tolerance 2e-2, achieved 5.4e-4). Halves
    DMA/SBUF traffic and enables 4-way fast-weight-load on LDWEIGHTS.
  - One input DMA per group: xt [128, 3, 1024] (2KB/partition rows).
  - qk: stationary [Wq|Wk] (M=128), moving xt pair-halves N=512 ->
    [q;k] stacked in PSUM; evacuated to rows 0:64 of zero-padded qz/kz.
  - v computed directly in [t, h] layout with xt chunks as stationary
    (no PE transposes); evacuated into v_aug with ones columns (softmax
    denominator) and zero padding to 128 cols (FWL-eligible outT loads).
  - weiT: K=128 zero-padded matmuls (rows 64:128 of qz/kz are zeros);
    the fully-masked (s-half1, t<128) block is never computed: per batch
    psw [128, 384] = s1-diag [128] + s0-full [256].
  - Softmax: one exp per batch (ACT, scale folded in, no max-subtraction
    needed: |wei/19.6| < ~4), one causal-mask multiply (GpSimd).
  - outT = [v|1|0]^T P per s-half (row 64 = denominator; the s-half1
    matmul only covers t >= 128). Normalization (divide by denom row)
    happens on the host during the gather, killing the per-batch
    reciprocal + broadcast matmul + multiply of v1.
  - Engine balance per group: PE ~3.7us (qk 6xN512, v 24xN64, weiT 8,
    outT 8), ACT ~3.3us (2 q-evacs + 4 exps), DVE ~3.4us (2 k-evacs +
    v-evac + 2 out-evacs), GpSimd ~2.7us (4 masks), all PSUM pools
    double-buffered inside the 8-bank budget.
"""

import os
from contextlib import ExitStack

import numpy as np

import concourse.bass as bass
import concourse.bacc as bacc
import concourse.tile as tile
from concourse import mybir
from concourse.bass_utils import run_bass_kernel_spmd

N_CORES = 8
B = 1024
T = 256
C = 384
H = 64
BPC = B // N_CORES  # 128 batches per core
GRP = 4  # batches per group
NG = BPC // GRP  # 32 groups per core
NCHUNK = C // 128  # 3
SCALE = float(C) ** -0.5

F32 = mybir.dt.float32
F16 = mybir.dt.float16
Exp = mybir.ActivationFunctionType.Exp


def _weiT(nc, psw_pool, s1, qz, kz, b):
    """wei^T for batch b via K=128 zero-padded matmuls into psw [128, 384]:
    cols 0:128 = wei[s 128:256, t 128:256], cols 128:384 = wei[s 0:128, t]."""
    j, bb = divmod(b, 2)
    base = j * 512 + bb * T
    psw = psw_pool.tile([128, 384], mybir.dt.float32, tag="psw", name="psw")
    nc.tensor.matmul(
        psw[:, 0:128],
        lhsT=kz[:, base + 128 : base + T],
        rhs=qz[:, base + 128 : base + T],
        start=True,
        stop=True,
    )
    nc.tensor.matmul(
        psw[:, 128:384],
        lhsT=kz[:, base : base + 128],
        rhs=qz[:, base : base + T],
        start=True,
        stop=True,
    )
    s1["psw"][b] = psw


def _softmax(nc, p0_pool, s1, mask_sb, b):
    p0 = p0_pool.tile([128, 384], F16, tag="p0", name="p0")
    nc.scalar.activation(p0[:], s1["psw"][b][:], Exp, scale=SCALE)
    nc.gpsimd.tensor_mul(p0[:, 0:256], p0[:, 0:256], mask_sb[:, 0:256])
    s1["p0"][b] = p0


def build_nc(ng: int = NG):
    nc = bacc.Bacc(
        "TRN2", target_bir_lowering=False, debug=False, num_devices=N_CORES
    )

    xt = nc.dram_tensor("xt", [ng, 128, NCHUNK, GRP * T], F16, kind="ExternalInput").ap()
    wqk = nc.dram_tensor("wqk", [128, NCHUNK, 128], F16, kind="ExternalInput").ap()
    wv = nc.dram_tensor("wv", [128, NCHUNK, H], F16, kind="ExternalInput").ap()
    mask = nc.dram_tensor("mask", [128, 512], F16, kind="ExternalInput").ap()
    outT = nc.dram_tensor("outT", [ng, H + 1, GRP * T], F16, kind="ExternalOutput").ap()

    with ExitStack() as ctx:
        tc = ctx.enter_context(tile.TileContext(nc))

        const = ctx.enter_context(tc.tile_pool(name="const", bufs=1))
        wqk_sb = const.tile([128, NCHUNK, 128], F16, tag="wqk")
        nc.sync.dma_start(wqk_sb[:], wqk)
        wv_sb = const.tile([128, NCHUNK, H], F16, tag="wv")
        nc.sync.dma_start(wv_sb[:], wv)
        mask_sb = const.tile([128, 512], F16, tag="mask")
        nc.sync.dma_start(mask_sb[:], mask)

        # Persistent tiles.
        # v_aug slots: per batch [128, 256]: cols 0:64 = v s-half0, col 64 =
        # ones (denominator row), cols 65:128 = ZERO pad (so the outT
        # LDWEIGHTS is a 128-column load -> fast-weight-load kicks in),
        # cols 128:192 = v s-half1, col 192 = ones, 193:256 = zero pad.
        vaug = []
        for i in range(4):
            v_t = const.tile([128, GRP, 256], F16, tag=f"vaug{i}")
            nc.gpsimd.memset(v_t[:], 0.0)
            nc.gpsimd.memset(v_t[:, :, 64:65], 1.0)
            nc.gpsimd.memset(v_t[:, :, 192:193], 1.0)
            vaug.append(v_t)
        # q/k slots for the K=128 zero-padded weiT: rows 0:64 hold q (k),
        # rows 64:128 stay zero so the padded contraction adds nothing.
        qzs, kzs = [], []
        for i in range(3):
            q_t = const.tile([128, 1024], F16, tag=f"qz{i}")
            nc.gpsimd.memset(q_t[64:128, :], 0.0)
            qzs.append(q_t)
            k_t = const.tile([128, 1024], F16, tag=f"kz{i}")
            nc.gpsimd.memset(k_t[64:128, :], 0.0)
            kzs.append(k_t)

        xt_pool = ctx.enter_context(tc.tile_pool(name="xt", bufs=4))
        p0_pool = ctx.enter_context(tc.tile_pool(name="p0", bufs=16))
        o_pool = ctx.enter_context(tc.tile_pool(name="o", bufs=2))
        psqk_pool = ctx.enter_context(tc.tile_pool(name="psqk", bufs=2, space="PSUM"))
        psv_pool = ctx.enter_context(tc.tile_pool(name="psv", bufs=2, space="PSUM"))
        psw_pool = ctx.enter_context(tc.tile_pool(name="psw", bufs=2, space="PSUM"))
        pso_pool = ctx.enter_context(tc.tile_pool(name="pso", bufs=2, space="PSUM"))

        # Per-group state carried between pipeline stages.
        st = {}

        for i in range(ng + 2):
            g0 = i  # stage 0: input DMA
            g1 = i - 1  # stage 1: qk/v matmuls, evacuations, weiT, softmax
            g3 = i - 3  # stage 3: outT, output evac + DMA (P is 2 iters old)

            if g0 < ng:
                xt_sb = xt_pool.tile([128, NCHUNK, GRP * T], F16, tag="xt")
                nc.sync.dma_start(xt_sb[:], xt[g0])
                st[g0] = {"xt": xt_sb}

            if 0 <= g1 < ng:
                s1 = st[g1]
                xt_sb = s1["xt"]
                qz, kz = qzs[g1 % 3], kzs[g1 % 3]
                s1["qz"], s1["kz"] = qz, kz
                # qk: per pair j, 3 accumulating MMs N=512 -> [q;k]
                # stacked. Evacuate q to rows 0:64 of the qz slot (cols
                # j*512+), k into kz; rows 64:128 are persistent zeros.
                for j in range(2):
                    psqk = psqk_pool.tile([128, 512], F32, tag="psqk")
                    for c in range(NCHUNK):
                        nc.tensor.matmul(
                            psqk[:],
                            lhsT=wqk_sb[:, c, :],
                            rhs=xt_sb[:, c, j * 512 : (j + 1) * 512],
                            start=(c == 0),
                            stop=(c == NCHUNK - 1),
                        )
                    nc.scalar.copy(
                        qz[0:64, j * 512 : (j + 1) * 512], psqk[0:64, :]
                    )
                    nc.vector.tensor_copy(
                        kz[0:64, j * 512 : (j + 1) * 512], psqk[64:128, :]
                    )

                # v direct in [t, h] layout: stationary = xt chunk t-half,
                # moving = Wv chunk. 8 regions x 3 accumulating MMs, N=64.
                psv = psv_pool.tile([128, 512], F32, tag="psv")
                for b in range(GRP):
                    for th in range(2):
                        off = b * 128 + th * 64
                        toff = b * T + th * 128
                        for c in range(NCHUNK):
                            nc.tensor.matmul(
                                psv[:, off : off + 64],
                                lhsT=xt_sb[:, c, toff : toff + 128],
                                rhs=wv_sb[:, c, :],
                                start=(c == 0),
                                stop=(c == NCHUNK - 1),
                            )
                v4 = vaug[g1 % 4]
                dst = v4.rearrange("p b (two g) -> p b two g", two=2)[
                    :, :, :, 0:64
                ]
                vsrc = psv[:].rearrange("p (b two f) -> p b two f", b=GRP, two=2)
                nc.vector.tensor_copy(dst, vsrc)
                s1["v4"] = v4

            g3_list = [g3] if 0 <= g3 else []
            if i == ng + 1:
                g3_list.append(ng - 1)  # drain: fold the last group in
            for g3 in g3_list:
                s3 = st[g3]
                v4 = s3["v4"]
                # outT: per batch, 2 accumulating MMs (s-halves). lhsT is the
                # 128-col padded v_aug slice: M=128 (rows 65:128 of the
                # output are zeros), col 64 = ones -> row 64 = softmax denom.
                # P is two iterations old: the exp->mask chain never stalls
                # these matmuls; they also fill this iteration's evac waits.
                pso = [
                    pso_pool.tile([128, 512], F32, tag="pso", name="pso")
                    for _ in range(2)
                ]
                for b in range(GRP):
                    j, bb = divmod(b, 2)
                    nc.tensor.matmul(
                        pso[j][:, bb * T : (bb + 1) * T],
                        lhsT=v4[:, b, 0:128],
                        rhs=s3["p0"][b][:, 128:384],
                        start=True,
                        stop=False,
                    )
                    # s-half1 contributes only to t >= 128 (causal).
                    nc.tensor.matmul(
                        pso[j][:, bb * T + 128 : (bb + 1) * T],
                        lhsT=v4[:, b, 128:256],
                        rhs=s3["p0"][b][:, 0:128],
                        start=False,
                        stop=True,
                    )

                o_sb = o_pool.tile([H + 1, GRP * T], F16, tag="o")
                nc.vector.tensor_copy(o_sb[:, 0 : 2 * T], pso[0][0 : H + 1, :])
                nc.vector.tensor_copy(o_sb[:, 2 * T : 4 * T], pso[1][0 : H + 1, :])
                nc.sync.dma_start(outT[g3], o_sb[:])
                del st[g3]

            if 0 <= g1 < ng:
                s1 = st[g1]
                s1["psw"], s1["p0"] = {}, {}
                for b in range(GRP):
                    _weiT(nc, psw_pool, s1, s1["qz"], s1["kz"], b)
                for b in range(GRP):
                    _softmax(nc, p0_pool, s1, mask_sb, b)

    nc.finalize()
    return nc


def _host_inputs(x, Wq, Wk, Wv):
    B_, T_, C_ = x.shape
    assert (B_, T_, C_) == (B, T, C), (B_, T_, C_)
    # xh[g, p, c, 256*i + t] = x[4g+i, t, 128c+p]
    xh = np.ascontiguousarray(
        x.reshape(B // GRP, GRP, T, NCHUNK, 128)
        .transpose(0, 4, 3, 1, 2)
        .reshape(B // GRP, 128, NCHUNK, GRP * T)
        .astype(np.float16)
    )
    wqk_h = np.ascontiguousarray(
        np.concatenate([Wq, Wk], axis=1).reshape(NCHUNK, 128, 128).transpose(1, 0, 2),
        dtype=np.float16,
    )
    wv_h = np.ascontiguousarray(
        Wv.reshape(NCHUNK, 128, H).transpose(1, 0, 2), dtype=np.float16
    )
    tri = np.triu(np.ones((128, 128), dtype=np.float16))
    mask_h = np.ascontiguousarray(np.concatenate([tri, tri, tri, tri], axis=1))
    return xh, wqk_h, wv_h, mask_h


def _gather(results):
    """Concatenate per-core outT, normalize, and restore [B, T, H] fp32."""
    outT = np.concatenate(
        [results[i]["outT"] for i in range(N_CORES)], axis=0
    ).astype(np.float32)  # [B/GRP, 65, GRP*T]
    outT = outT.reshape(B // GRP, H + 1, GRP, T)
    out = outT[:, 0:H] / outT[:, H : H + 1]  # [B/GRP, H, GRP, T]
    return np.ascontiguousarray(
        out.transpose(0, 2, 3, 1).reshape(B, T, H).astype(np.float32)
    )


def kernel(x, Wq, Wk, Wv):
    x = np.asarray(x, dtype=np.float32)
    Wq = np.asarray(Wq, dtype=np.float32)
    Wk = np.asarray(Wk, dtype=np.float32)
    Wv = np.asarray(Wv, dtype=np.float32)

    xh, wqk_h, wv_h, mask_h = _host_inputs(x, Wq, Wk, Wv)

    nc = build_nc(NG)
    in_maps = [
        {
            "xt": xh[i * NG : (i + 1) * NG],
            "wqk": wqk_h,
            "wv": wv_h,
            "mask": mask_h,
        }
        for i in range(N_CORES)
    ]
    res = run_bass_kernel_spmd(nc, in_maps, list(range(N_CORES)))
    return _gather(res.results)


# revision 48
# speedup vs baseline: 1.1722x; 1.1722x over previous
"""Single-head causal self-attention on 8 Trainium2 NeuronCores (Bass/Tile).

Problem: x [1024, 256, 384], Wq/Wk/Wv [384, 64] ->
  q,k,v = x@W;  wei = softmax(mask(q k^T / sqrt(384)));  out = wei @ v
Output: [1024, 256, 64] fp32.

Design (4.3x over the fp32r v1 baseline: 661us -> ~155us):
  - Data-parallel over batch: 128 batches per core, processed as 32
    groups of 4 with a 4-deep software pipeline (DMA -> projections ->
    attention -> output, P matrices consumed two iterations after they
    are produced so the softmax chain never stalls the PE).
  - fp16 operands everywhere (tolerance 2e-2, achieved 5.4e-4). Halves
    DMA/SBUF traffic and enables 4-way fast-weight-load on LDWEIGHTS.
  - One input DMA per group: xt [128, 3, 1024] (2KB/partition rows).
  - qk: stationary [Wq|Wk] (M=128), moving xt pair-halves N=512 ->
    [q;k] stacked in PSUM; evacuated to rows 0:64 of zero-padded qz/kz.
  - v computed directly in [t, h] layout with xt chunks as stationary
    (no PE transposes); evacuated into v_aug with ones columns (softmax
    denominator) and zero padding to 128 cols (FWL-eligible outT loads).
  - weiT: K=128 zero-padded matmuls (rows 64:128 of qz/kz are zeros);
    the fully-masked (s-half1, t<128) block is never computed: per batch
    psw [128, 384] = s1-diag [128] + s0-full [256].
  - Softmax: one exp per batch (ACT, scale folded in, no max-subtraction
    needed: |wei/19.6| < ~4), one causal-mask multiply (GpSimd).
  - outT = [v|1|0]^T P per s-half (row 64 = denominator; the s-half1
    matmul only covers t >= 128). Normalization (divide by denom row)
    happens on the host during the gather, killing the per-batch
    reciprocal + broadcast matmul + multiply of v1.
  - Engine balance per group: PE ~3.7us (qk 6xN512, v 24xN64, weiT 8,
    outT 8), ACT ~3.3us (2 q-evacs + 4 exps), DVE ~3.4us (2 k-evacs +
    v-evac + 2 out-evacs), GpSimd ~2.7us (4 masks), all PSUM pools
    double-buffered inside the 8-bank budget.
"""

import os
from contextlib import ExitStack

import numpy as np

import concourse.bass as bass
import concourse.bacc as bacc
import concourse.tile as tile
from concourse import mybir
from concourse.bass_utils import run_bass_kernel_spmd

N_CORES = 8
B = 1024
T = 256
C = 384
H = 64
BPC = B // N_CORES  # 128 batches per core
GRP = 4  # batches per group
NG = BPC // GRP  # 32 groups per core
NCHUNK = C // 128  # 3
SCALE = float(C) ** -0.5

F32 = mybir.dt.float32
F16 = mybir.dt.float16
Exp = mybir.ActivationFunctionType.Exp


def _weiT(nc, psw_pool, s1, qz, kz, b):
    """wei^T for batch b via K=128 zero-padded matmuls into psw [128, 384]:
    cols 0:128 = wei[s 128:256, t 128:256], cols 128:384 = wei[s 0:128, t]."""
    j, bb = divmod(b, 2)
    base = j * 512 + bb * T
    psw = psw_pool.tile([128, 384], mybir.dt.float32, tag="psw", name="psw")
    nc.tensor.matmul(
        psw[:, 0:128],
        lhsT=kz[:, base + 128 : base + T],
        rhs=qz[:, base + 128 : base + T],
        start=True,
        stop=True,
    )
    nc.tensor.matmul(
        psw[:, 128:384],
        lhsT=kz[:, base : base + 128],
        rhs=qz[:, base : base + T],
        start=True,
        stop=True,
    )
    s1["psw"][b] = psw


def _softmax(nc, p0_pool, s1, mask_sb, b):
    p0 = p0_pool.tile([128, 384], F16, tag="p0", name="p0")
    nc.scalar.activation(p0[:], s1["psw"][b][:], Exp, scale=SCALE)
    nc.gpsimd.tensor_mul(p0[:, 0:256], p0[:, 0:256], mask_sb[:, 0:256])
    s1["p0"][b] = p0


def build_nc(ng: int = NG):
    nc = bacc.Bacc(
        "TRN2", target_bir_lowering=False, debug=False, num_devices=N_CORES
    )

    xt = nc.dram_tensor("xt", [ng, 128, NCHUNK, GRP * T], F16, kind="ExternalInput").ap()
    wqk = nc.dram_tensor("wqk", [128, NCHUNK, 128], F16, kind="ExternalInput").ap()
    wv = nc.dram_tensor("wv", [128, NCHUNK, H], F16, kind="ExternalInput").ap()
    mask = nc.dram_tensor("mask", [128, 512], F16, kind="ExternalInput").ap()
    outT = nc.dram_tensor("outT", [ng, H + 1, GRP * T], F16, kind="ExternalOutput").ap()

    with ExitStack() as ctx:
        tc = ctx.enter_context(tile.TileContext(nc))

        const = ctx.enter_context(tc.tile_pool(name="const", bufs=1))
        wqk_sb = const.tile([128, NCHUNK, 128], F16, tag="wqk")
        nc.sync.dma_start(wqk_sb[:], wqk)
        wv_sb = const.tile([128, NCHUNK, H], F16, tag="wv")
        nc.sync.dma_start(wv_sb[:], wv)
        mask_sb = const.tile([128, 512], F16, tag="mask")
        nc.sync.dma_start(mask_sb[:], mask)

        # Persistent tiles.
        # v_aug slots: per batch [128, 256]: cols 0:64 = v s-half0, col 64 =
        # ones (denominator row), cols 65:128 = ZERO pad (so the outT
        # LDWEIGHTS is a 128-column load -> fast-weight-load kicks in),
        # cols 128:192 = v s-half1, col 192 = ones, 193:256 = zero pad.
        vaug = []
        for i in range(4):
            v_t = const.tile([128, GRP, 256], F16, tag=f"vaug{i}")
            nc.gpsimd.memset(v_t[:], 0.0)
            nc.gpsimd.memset(v_t[:, :, 64:65], 1.0)
            nc.gpsimd.memset(v_t[:, :, 192:193], 1.0)
            vaug.append(v_t)
        # q/k slots for the K=128 zero-padded weiT: rows 0:64 hold q (k),
        # rows 64:128 stay zero so the padded contraction adds nothing.
        qzs, kzs = [], []
        for i in range(3):
            q_t = const.tile([128, 1024], F16, tag=f"qz{i}")
            nc.gpsimd.memset(q_t[64:128, :], 0.0)
            qzs.append(q_t)
            k_t = const.tile([128, 1024], F16, tag=f"kz{i}")
            nc.gpsimd.memset(k_t[64:128, :], 0.0)
            kzs.append(k_t)

        xt_pool = ctx.enter_context(tc.tile_pool(name="xt", bufs=4))
        p0_pool = ctx.enter_context(tc.tile_pool(name="p0", bufs=16))
        o_pool = ctx.enter_context(tc.tile_pool(name="o", bufs=2))
        psqk_pool = ctx.enter_context(tc.tile_pool(name="psqk", bufs=2, space="PSUM"))
        psv_pool = ctx.enter_context(tc.tile_pool(name="psv", bufs=2, space="PSUM"))
        psw_pool = ctx.enter_context(tc.tile_pool(name="psw", bufs=2, space="PSUM"))
        pso_pool = ctx.enter_context(tc.tile_pool(name="pso", bufs=2, space="PSUM"))

        # Per-group state carried between pipeline stages.
        st = {}

        for i in range(ng + 2):
            g0 = i  # stage 0: input DMA
            g1 = i - 1  # stage 1: qk/v matmuls, evacuations, weiT, softmax
            g3 = i - 3  # stage 3: outT, output evac + DMA (P is 2 iters old)

            if g0 < ng:
                xt_sb = xt_pool.tile([128, NCHUNK, GRP * T], F16, tag="xt")
                if i == 0:
                    # Prologue: split the first DMA so the very first qk
                    # matmuls (which read cols 0:512) start half a transfer
                    # earlier.
                    nc.sync.dma_start(xt_sb[:, :, 0:512], xt[g0][:, :, 0:512])
                    nc.sync.dma_start(
                        xt_sb[:, :, 512:1024], xt[g0][:, :, 512:1024]
                    )
                else:
                    nc.sync.dma_start(xt_sb[:], xt[g0])
                st[g0] = {"xt": xt_sb}

            if 0 <= g1 < ng:
                s1 = st[g1]
                xt_sb = s1["xt"]
                qz, kz = qzs[g1 % 3], kzs[g1 % 3]
                s1["qz"], s1["kz"] = qz, kz
                # qk: per pair j, 3 accumulating MMs N=512 -> [q;k]
                # stacked. Evacuate q to rows 0:64 of the qz slot (cols
                # j*512+), k into kz; rows 64:128 are persistent zeros.
                for j in range(2):
                    psqk = psqk_pool.tile([128, 512], F32, tag="psqk")
                    for c in range(NCHUNK):
                        nc.tensor.matmul(
                            psqk[:],
                            lhsT=wqk_sb[:, c, :],
                            rhs=xt_sb[:, c, j * 512 : (j + 1) * 512],
                            start=(c == 0),
                            stop=(c == NCHUNK - 1),
                        )
                    nc.scalar.copy(
                        qz[0:64, j * 512 : (j + 1) * 512], psqk[0:64, :]
                    )
                    nc.vector.tensor_copy(
                        kz[0:64, j * 512 : (j + 1) * 512], psqk[64:128, :]
                    )

                # v direct in [t, h] layout: stationary = xt chunk t-half,
                # moving = Wv chunk. 8 regions x 3 accumulating MMs, N=64.
                psv = psv_pool.tile([128, 512], F32, tag="psv")
                for b in range(GRP):
                    for th in range(2):
                        off = b * 128 + th * 64
                        toff = b * T + th * 128
                        for c in range(NCHUNK):
                            nc.tensor.matmul(
                                psv[:, off : off + 64],
                                lhsT=xt_sb[:, c, toff : toff + 128],
                                rhs=wv_sb[:, c, :],
                                start=(c == 0),
                                stop=(c == NCHUNK - 1),
                            )
                v4 = vaug[g1 % 4]
                dst = v4.rearrange("p b (two g) -> p b two g", two=2)[
                    :, :, :, 0:64
                ]
                vsrc = psv[:].rearrange("p (b two f) -> p b two f", b=GRP, two=2)
                nc.vector.tensor_copy(dst, vsrc)
                s1["v4"] = v4

            g3_list = [g3] if 0 <= g3 else []
            if i == ng + 1:
                g3_list.append(ng - 1)  # drain: fold the last group in
            for g3 in g3_list:
                s3 = st[g3]
                v4 = s3["v4"]
                # outT: per batch, 2 accumulating MMs (s-halves). lhsT is the
                # 128-col padded v_aug slice: M=128 (rows 65:128 of the
                # output are zeros), col 64 = ones -> row 64 = softmax denom.
                # P is two iterations old: the exp->mask chain never stalls
                # these matmuls; they also fill this iteration's evac waits.
                pso = [
                    pso_pool.tile([128, 512], F32, tag="pso", name="pso")
                    for _ in range(2)
                ]
                for b in range(GRP):
                    j, bb = divmod(b, 2)
                    nc.tensor.matmul(
                        pso[j][:, bb * T : (bb + 1) * T],
                        lhsT=v4[:, b, 0:128],
                        rhs=s3["p0"][b][:, 128:384],
                        start=True,
                        stop=False,
                    )
                    # s-half1 contributes only to t >= 128 (causal).
                    nc.tensor.matmul(
                        pso[j][:, bb * T + 128 : (bb + 1) * T],
                        lhsT=v4[:, b, 128:256],
                        rhs=s3["p0"][b][:, 0:128],
                        start=False,
                        stop=True,
                    )

                o_sb = o_pool.tile([H + 1, GRP * T], F16, tag="o")
                nc.vector.tensor_copy(o_sb[:, 0 : 2 * T], pso[0][0 : H + 1, :])
                nc.vector.tensor_copy(o_sb[:, 2 * T : 4 * T], pso[1][0 : H + 1, :])
                nc.sync.dma_start(outT[g3], o_sb[:])
                del st[g3]

            if 0 <= g1 < ng:
                s1 = st[g1]
                s1["psw"], s1["p0"] = {}, {}
                for b in range(GRP):
                    _weiT(nc, psw_pool, s1, s1["qz"], s1["kz"], b)
                for b in range(GRP):
                    _softmax(nc, p0_pool, s1, mask_sb, b)

    nc.finalize()
    return nc


def _host_inputs(x, Wq, Wk, Wv):
    B_, T_, C_ = x.shape
    assert (B_, T_, C_) == (B, T, C), (B_, T_, C_)
    # xh[g, p, c, 256*i + t] = x[4g+i, t, 128c+p]
    xh = np.ascontiguousarray(
        x.reshape(B // GRP, GRP, T, NCHUNK, 128)
        .transpose(0, 4, 3, 1, 2)
        .reshape(B // GRP, 128, NCHUNK, GRP * T)
        .astype(np.float16)
    )
    wqk_h = np.ascontiguousarray(
        np.concatenate([Wq, Wk], axis=1).reshape(NCHUNK, 128, 128).transpose(1, 0, 2),
        dtype=np.float16,
    )
    wv_h = np.ascontiguousarray(
        Wv.reshape(NCHUNK, 128, H).transpose(1, 0, 2), dtype=np.float16
    )
    tri = np.triu(np.ones((128, 128), dtype=np.float16))
    mask_h = np.ascontiguousarray(np.concatenate([tri, tri, tri, tri], axis=1))
    return xh, wqk_h, wv_h, mask_h


def _gather(results):
    """Concatenate per-core outT, normalize, and restore [B, T, H] fp32."""
    outT = np.concatenate(
        [results[i]["outT"] for i in range(N_CORES)], axis=0
    ).astype(np.float32)  # [B/GRP, 65, GRP*T]
    outT = outT.reshape(B // GRP, H + 1, GRP, T)
    out = outT[:, 0:H] / outT[:, H : H + 1]  # [B/GRP, H, GRP, T]
    return np.ascontiguousarray(
        out.transpose(0, 2, 3, 1).reshape(B, T, H).astype(np.float32)
    )


def kernel(x, Wq, Wk, Wv):
    x = np.asarray(x, dtype=np.float32)
    Wq = np.asarray(Wq, dtype=np.float32)
    Wk = np.asarray(Wk, dtype=np.float32)
    Wv = np.asarray(Wv, dtype=np.float32)

    xh, wqk_h, wv_h, mask_h = _host_inputs(x, Wq, Wk, Wv)

    nc = build_nc(NG)
    in_maps = [
        {
            "xt": xh[i * NG : (i + 1) * NG],
            "wqk": wqk_h,
            "wv": wv_h,
            "mask": mask_h,
        }
        for i in range(N_CORES)
    ]
    res = run_bass_kernel_spmd(nc, in_maps, list(range(N_CORES)))
    return _gather(res.results)


# revision 49
# speedup vs baseline: 1.2368x; 1.0551x over previous
"""Single-head causal self-attention on 8 Trainium2 NeuronCores (Bass/Tile).

Problem: x [1024, 256, 384], Wq/Wk/Wv [384, 64] ->
  q,k,v = x@W;  wei = softmax(mask(q k^T / sqrt(384)));  out = wei @ v
Output: [1024, 256, 64] fp32.

Design (4.3x over the fp32r v1 baseline: 661us -> ~155us):
  - Data-parallel over batch: 128 batches per core, processed as 32
    groups of 4 with a 4-deep software pipeline (DMA -> projections ->
    attention -> output, P matrices consumed two iterations after they
    are produced so the softmax chain never stalls the PE).
  - fp16 operands everywhere (tolerance 2e-2, achieved 5.4e-4). Halves
    DMA/SBUF traffic and enables 4-way fast-weight-load on LDWEIGHTS.
  - One input DMA per group: xt [128, 3, 1024] (2KB/partition rows).
  - qk: stationary [Wq|Wk] (M=128), moving xt pair-halves N=512 ->
    [q;k] stacked in PSUM; evacuated to rows 0:64 of zero-padded qz/kz.
  - v computed directly in [t, h] layout with xt chunks as stationary
    (no PE transposes); evacuated into v_aug with ones columns (softmax
    denominator) and zero padding to 128 cols (FWL-eligible outT loads).
  - weiT: K=128 zero-padded matmuls (rows 64:128 of qz/kz are zeros);
    the fully-masked (s-half1, t<128) block is never computed: per batch
    psw [128, 384] = s1-diag [128] + s0-full [256].
  - Softmax: one exp per batch (ACT, scale folded in, no max-subtraction
    needed: |wei/19.6| < ~4), one causal-mask multiply (GpSimd).
  - outT = [v|1|0]^T P per s-half (row 64 = denominator; the s-half1
    matmul only covers t >= 128). Normalization (divide by denom row)
    happens on the host during the gather, killing the per-batch
    reciprocal + broadcast matmul + multiply of v1.
  - Engine balance per group: PE ~3.7us (qk 6xN512, v 24xN64, weiT 8,
    outT 8), ACT ~3.3us (2 q-evacs + 4 exps), DVE ~3.4us (2 k-evacs +
    v-evac + 2 out-evacs), GpSimd ~2.7us (4 masks), all PSUM pools
    double-buffered inside the 8-bank budget.
"""

import os
from contextlib import ExitStack

import numpy as np

import concourse.bass as bass
import concourse.bacc as bacc
import concourse.tile as tile
from concourse import mybir
from concourse.bass_utils import run_bass_kernel_spmd

N_CORES = 8
B = 1024
T = 256
C = 384
H = 64
BPC = B // N_CORES  # 128 batches per core
GRP = 4  # batches per group
NG = BPC // GRP  # 32 groups per core
NCHUNK = C // 128  # 3
SCALE = float(C) ** -0.5

F32 = mybir.dt.float32
F16 = mybir.dt.float16
Exp = mybir.ActivationFunctionType.Exp


def _weiT(nc, psw_pool, s1, qz, kz, b):
    """wei^T for batch b via K=128 zero-padded matmuls into psw [128, 384]:
    cols 0:128 = wei[s 128:256, t 128:256], cols 128:384 = wei[s 0:128, t]."""
    j, bb = divmod(b, 2)
    base = j * 512 + bb * T
    psw = psw_pool.tile([128, 384], mybir.dt.float32, tag="psw", name="psw")
    nc.tensor.matmul(
        psw[:, 0:128],
        lhsT=kz[:, base + 128 : base + T],
        rhs=qz[:, base + 128 : base + T],
        start=True,
        stop=True,
    )
    nc.tensor.matmul(
        psw[:, 128:384],
        lhsT=kz[:, base : base + 128],
        rhs=qz[:, base : base + T],
        start=True,
        stop=True,
    )
    s1["psw"][b] = psw


def _softmax(nc, p0_pool, s1, mask_sb, b):
    p0 = p0_pool.tile([128, 384], F16, tag="p0", name="p0")
    nc.scalar.activation(p0[:], s1["psw"][b][:], Exp, scale=SCALE)
    nc.gpsimd.tensor_mul(p0[:, 0:256], p0[:, 0:256], mask_sb[:, 0:256])
    s1["p0"][b] = p0


def build_nc(ng: int = NG):
    nc = bacc.Bacc(
        "TRN2", target_bir_lowering=False, debug=False, num_devices=N_CORES
    )

    xt = nc.dram_tensor("xt", [ng, 128, NCHUNK, GRP * T], F16, kind="ExternalInput").ap()
    wqk = nc.dram_tensor("wqk", [128, NCHUNK, 128], F16, kind="ExternalInput").ap()
    wv = nc.dram_tensor("wv", [128, NCHUNK, H], F16, kind="ExternalInput").ap()
    mask = nc.dram_tensor("mask", [128, 512], F16, kind="ExternalInput").ap()
    outT = nc.dram_tensor("outT", [ng, H + 1, GRP * T], F16, kind="ExternalOutput").ap()

    with ExitStack() as ctx:
        tc = ctx.enter_context(tile.TileContext(nc))

        const = ctx.enter_context(tc.tile_pool(name="const", bufs=1))
        wqk_sb = const.tile([128, NCHUNK, 128], F16, tag="wqk")
        nc.sync.dma_start(wqk_sb[:], wqk)
        wv_sb = const.tile([128, NCHUNK, H], F16, tag="wv")
        nc.sync.dma_start(wv_sb[:], wv)
        mask_sb = const.tile([128, 512], F16, tag="mask")
        nc.sync.dma_start(mask_sb[:], mask)

        # Persistent tiles.
        # v_aug slots: per batch [128, 256]: cols 0:64 = v s-half0, col 64 =
        # ones (denominator row), cols 65:128 = ZERO pad (so the outT
        # LDWEIGHTS is a 128-column load -> fast-weight-load kicks in),
        # cols 128:192 = v s-half1, col 192 = ones, 193:256 = zero pad.
        vaug = []
        for i in range(4):
            v_t = const.tile([128, GRP, 256], F16, tag=f"vaug{i}")
            nc.gpsimd.memset(v_t[:], 0.0)
            nc.gpsimd.memset(v_t[:, :, 64:65], 1.0)
            nc.gpsimd.memset(v_t[:, :, 192:193], 1.0)
            vaug.append(v_t)
        # q/k slots for the K=128 zero-padded weiT: rows 0:64 hold q (k),
        # rows 64:128 stay zero so the padded contraction adds nothing.
        qzs, kzs = [], []
        for i in range(3):
            q_t = const.tile([128, 1024], F16, tag=f"qz{i}")
            nc.gpsimd.memset(q_t[64:128, :], 0.0)
            qzs.append(q_t)
            k_t = const.tile([128, 1024], F16, tag=f"kz{i}")
            nc.gpsimd.memset(k_t[64:128, :], 0.0)
            kzs.append(k_t)

        xt_pool = ctx.enter_context(tc.tile_pool(name="xt", bufs=4))
        p0_pool = ctx.enter_context(tc.tile_pool(name="p0", bufs=16))
        o_pool = ctx.enter_context(tc.tile_pool(name="o", bufs=3))
        psqk_pool = ctx.enter_context(tc.tile_pool(name="psqk", bufs=2, space="PSUM"))
        psv_pool = ctx.enter_context(tc.tile_pool(name="psv", bufs=2, space="PSUM"))
        psw_pool = ctx.enter_context(tc.tile_pool(name="psw", bufs=2, space="PSUM"))
        pso_pool = ctx.enter_context(tc.tile_pool(name="pso", bufs=2, space="PSUM"))

        # Per-group state carried between pipeline stages.
        st = {}

        for i in range(ng + 2):
            g0 = i  # stage 0: input DMA
            g1 = i - 1  # stage 1: qk/v matmuls, evacuations, weiT, softmax
            g3 = i - 3  # stage 3: outT, output evac + DMA (P is 2 iters old)

            if g0 < ng:
                xt_sb = xt_pool.tile([128, NCHUNK, GRP * T], F16, tag="xt")
                if i == 0:
                    # Prologue: split the first DMA so the very first qk
                    # matmuls (which read cols 0:512) start half a transfer
                    # earlier.
                    nc.sync.dma_start(xt_sb[:, :, 0:512], xt[g0][:, :, 0:512])
                    nc.sync.dma_start(
                        xt_sb[:, :, 512:1024], xt[g0][:, :, 512:1024]
                    )
                else:
                    nc.sync.dma_start(xt_sb[:], xt[g0])
                st[g0] = {"xt": xt_sb}

            if 0 <= g1 < ng:
                s1 = st[g1]
                xt_sb = s1["xt"]
                qz, kz = qzs[g1 % 3], kzs[g1 % 3]
                s1["qz"], s1["kz"] = qz, kz
                # qk: per pair j, 3 accumulating MMs N=512 -> [q;k]
                # stacked. Evacuate q to rows 0:64 of the qz slot (cols
                # j*512+), k into kz; rows 64:128 are persistent zeros.
                for j in range(2):
                    psqk = psqk_pool.tile([128, 512], F32, tag="psqk")
                    for c in range(NCHUNK):
                        nc.tensor.matmul(
                            psqk[:],
                            lhsT=wqk_sb[:, c, :],
                            rhs=xt_sb[:, c, j * 512 : (j + 1) * 512],
                            start=(c == 0),
                            stop=(c == NCHUNK - 1),
                        )
                    nc.scalar.copy(
                        qz[0:64, j * 512 : (j + 1) * 512], psqk[0:64, :]
                    )
                    nc.vector.tensor_copy(
                        kz[0:64, j * 512 : (j + 1) * 512], psqk[64:128, :]
                    )

                # v direct in [t, h] layout: stationary = xt chunk t-half,
                # moving = Wv chunk. 8 regions x 3 accumulating MMs, N=64.
                psv = psv_pool.tile([128, 512], F32, tag="psv")
                for b in range(GRP):
                    for th in range(2):
                        off = b * 128 + th * 64
                        toff = b * T + th * 128
                        for c in range(NCHUNK):
                            nc.tensor.matmul(
                                psv[:, off : off + 64],
                                lhsT=xt_sb[:, c, toff : toff + 128],
                                rhs=wv_sb[:, c, :],
                                start=(c == 0),
                                stop=(c == NCHUNK - 1),
                            )
                v4 = vaug[g1 % 4]
                dst = v4.rearrange("p b (two g) -> p b two g", two=2)[
                    :, :, :, 0:64
                ]
                vsrc = psv[:].rearrange("p (b two f) -> p b two f", b=GRP, two=2)
                nc.vector.tensor_copy(dst, vsrc)
                s1["v4"] = v4

            g3_list = [g3] if 0 <= g3 else []
            if i == ng + 1:
                g3_list.append(ng - 1)  # drain: fold the last group in
            for g3 in g3_list:
                s3 = st[g3]
                v4 = s3["v4"]
                # outT: per batch, 2 accumulating MMs (s-halves). lhsT is the
                # 128-col padded v_aug slice: M=128 (rows 65:128 of the
                # output are zeros), col 64 = ones -> row 64 = softmax denom.
                # P is two iterations old: the exp->mask chain never stalls
                # these matmuls; they also fill this iteration's evac waits.
                pso = [
                    pso_pool.tile([128, 512], F32, tag="pso", name="pso")
                    for _ in range(2)
                ]
                for b in range(GRP):
                    j, bb = divmod(b, 2)
                    nc.tensor.matmul(
                        pso[j][:, bb * T : (bb + 1) * T],
                        lhsT=v4[:, b, 0:128],
                        rhs=s3["p0"][b][:, 128:384],
                        start=True,
                        stop=False,
                    )
                    # s-half1 contributes only to t >= 128 (causal).
                    nc.tensor.matmul(
                        pso[j][:, bb * T + 128 : (bb + 1) * T],
                        lhsT=v4[:, b, 128:256],
                        rhs=s3["p0"][b][:, 0:128],
                        start=False,
                        stop=True,
                    )

                o_sb = o_pool.tile([H + 1, GRP * T], F16, tag="o")
                nc.vector.tensor_copy(o_sb[:, 0 : 2 * T], pso[0][0 : H + 1, :])
                nc.vector.tensor_copy(o_sb[:, 2 * T : 4 * T], pso[1][0 : H + 1, :])
                nc.sync.dma_start(outT[g3], o_sb[:])
                del st[g3]

            if 0 <= g1 < ng:
                s1 = st[g1]
                s1["psw"], s1["p0"] = {}, {}
                for b in range(GRP):
                    _weiT(nc, psw_pool, s1, s1["qz"], s1["kz"], b)
                for b in range(GRP):
                    _softmax(nc, p0_pool, s1, mask_sb, b)

    nc.finalize()
    return nc


def _host_inputs(x, Wq, Wk, Wv):
    B_, T_, C_ = x.shape
    assert (B_, T_, C_) == (B, T, C), (B_, T_, C_)
    # xh[g, p, c, 256*i + t] = x[4g+i, t, 128c+p]
    xh = np.ascontiguousarray(
        x.reshape(B // GRP, GRP, T, NCHUNK, 128)
        .transpose(0, 4, 3, 1, 2)
        .reshape(B // GRP, 128, NCHUNK, GRP * T)
        .astype(np.float16)
    )
    wqk_h = np.ascontiguousarray(
        np.concatenate([Wq, Wk], axis=1).reshape(NCHUNK, 128, 128).transpose(1, 0, 2),
        dtype=np.float16,
    )
    wv_h = np.ascontiguousarray(
        Wv.reshape(NCHUNK, 128, H).transpose(1, 0, 2), dtype=np.float16
    )
    tri = np.triu(np.ones((128, 128), dtype=np.float16))
    mask_h = np.ascontiguousarray(np.concatenate([tri, tri, tri, tri], axis=1))
    return xh, wqk_h, wv_h, mask_h


def _gather(results):
    """Concatenate per-core outT, normalize, and restore [B, T, H] fp32."""
    outT = np.concatenate(
        [results[i]["outT"] for i in range(N_CORES)], axis=0
    ).astype(np.float32)  # [B/GRP, 65, GRP*T]
    outT = outT.reshape(B // GRP, H + 1, GRP, T)
    out = outT[:, 0:H] / outT[:, H : H + 1]  # [B/GRP, H, GRP, T]
    return np.ascontiguousarray(
        out.transpose(0, 2, 3, 1).reshape(B, T, H).astype(np.float32)
    )


def kernel(x, Wq, Wk, Wv):
    x = np.asarray(x, dtype=np.float32)
    Wq = np.asarray(Wq, dtype=np.float32)
    Wk = np.asarray(Wk, dtype=np.float32)
    Wv = np.asarray(Wv, dtype=np.float32)

    xh, wqk_h, wv_h, mask_h = _host_inputs(x, Wq, Wk, Wv)

    nc = build_nc(NG)
    in_maps = [
        {
            "xt": xh[i * NG : (i + 1) * NG],
            "wqk": wqk_h,
            "wv": wv_h,
            "mask": mask_h,
        }
        for i in range(N_CORES)
    ]
    res = run_bass_kernel_spmd(nc, in_maps, list(range(N_CORES)))
    return _gather(res.results)


# revision 50
# speedup vs baseline: 1.2424x; 1.0045x over previous
"""Single-head causal self-attention on 8 Trainium2 NeuronCores (Bass/Tile).

Problem: x [1024, 256, 384], Wq/Wk/Wv [384, 64] ->
  q,k,v = x@W;  wei = softmax(mask(q k^T / sqrt(384)));  out = wei @ v
Output: [1024, 256, 64] fp32.

Design (4.3x over the fp32r v1 baseline: 661us -> ~155us):
  - Data-parallel over batch: 128 batches per core, processed as 32
    groups of 4 with a 4-deep software pipeline (DMA -> projections ->
    attention -> output, P matrices consumed two iterations after they
    are produced so the softmax chain never stalls the PE).
  - fp16 operands everywhere (tolerance 2e-2, achieved 5.4e-4). Halves
    DMA/SBUF traffic and enables 4-way fast-weight-load on LDWEIGHTS.
  - One input DMA per group: xt [128, 3, 1024] (2KB/partition rows).
  - qk: stationary [Wq|Wk] (M=128), moving xt pair-halves N=512 ->
    [q;k] stacked in PSUM; evacuated to rows 0:64 of zero-padded qz/kz.
  - v computed directly in [t, h] layout with xt chunks as stationary
    (no PE transposes); evacuated into v_aug with ones columns (softmax
    denominator) and zero padding to 128 cols (FWL-eligible outT loads).
  - weiT: K=128 zero-padded matmuls (rows 64:128 of qz/kz are zeros);
    the fully-masked (s-half1, t<128) block is never computed: per batch
    psw [128, 384] = s1-diag [128] + s0-full [256].
  - Softmax: one exp per batch (ACT, scale folded in, no max-subtraction
    needed: |wei/19.6| < ~4), one causal-mask multiply (GpSimd).
  - outT = [v|1|0]^T P per s-half (row 64 = denominator; the s-half1
    matmul only covers t >= 128). Normalization (divide by denom row)
    happens on the host during the gather, killing the per-batch
    reciprocal + broadcast matmul + multiply of v1.
  - Engine balance per group: PE ~3.7us (qk 6xN512, v 24xN64, weiT 8,
    outT 8), ACT ~3.3us (2 q-evacs + 4 exps), DVE ~3.4us (2 k-evacs +
    v-evac + 2 out-evacs), GpSimd ~2.7us (4 masks), all PSUM pools
    double-buffered inside the 8-bank budget.
"""

import os
from contextlib import ExitStack

import numpy as np

import concourse.bass as bass
import concourse.bacc as bacc
import concourse.tile as tile
from concourse import mybir
from concourse.bass_utils import run_bass_kernel_spmd

N_CORES = 8
B = 1024
T = 256
C = 384
H = 64
BPC = B // N_CORES  # 128 batches per core
GRP = 4  # batches per group
NG = BPC // GRP  # 32 groups per core
NCHUNK = C // 128  # 3
SCALE = float(C) ** -0.5

F32 = mybir.dt.float32
F16 = mybir.dt.float16
Exp = mybir.ActivationFunctionType.Exp


def _weiT(nc, psw_pool, s1, qz, kz, b):
    """wei^T for batch b via K=128 zero-padded matmuls into psw [128, 384]:
    cols 0:128 = wei[s 128:256, t 128:256], cols 128:384 = wei[s 0:128, t]."""
    j, bb = divmod(b, 2)
    base = j * 512 + bb * T
    psw = psw_pool.tile([128, 384], mybir.dt.float32, tag="psw", name="psw")
    nc.tensor.matmul(
        psw[:, 0:128],
        lhsT=kz[:, base + 128 : base + T],
        rhs=qz[:, base + 128 : base + T],
        start=True,
        stop=True,
    )
    nc.tensor.matmul(
        psw[:, 128:384],
        lhsT=kz[:, base : base + 128],
        rhs=qz[:, base : base + T],
        start=True,
        stop=True,
    )
    s1["psw"][b] = psw


def _softmax(nc, p0_pool, s1, mask_sb, b):
    p0 = p0_pool.tile([128, 384], F16, tag="p0", name="p0")
    nc.scalar.activation(p0[:], s1["psw"][b][:], Exp, scale=SCALE)
    nc.gpsimd.tensor_mul(p0[:, 0:256], p0[:, 0:256], mask_sb[:, 0:256])
    s1["p0"][b] = p0


def build_nc(ng: int = NG):
    nc = bacc.Bacc(
        "TRN2", target_bir_lowering=False, debug=False, num_devices=N_CORES
    )

    xt = nc.dram_tensor("xt", [ng, 128, NCHUNK, GRP * T], F16, kind="ExternalInput").ap()
    wqk = nc.dram_tensor("wqk", [128, NCHUNK, 128], F16, kind="ExternalInput").ap()
    wv = nc.dram_tensor("wv", [128, NCHUNK, H], F16, kind="ExternalInput").ap()
    mask = nc.dram_tensor("mask", [128, 512], F16, kind="ExternalInput").ap()
    outT = nc.dram_tensor("outT", [ng, H + 1, GRP * T], F16, kind="ExternalOutput").ap()

    with ExitStack() as ctx:
        tc = ctx.enter_context(tile.TileContext(nc))

        const = ctx.enter_context(tc.tile_pool(name="const", bufs=1))
        wqk_sb = const.tile([128, NCHUNK, 128], F16, tag="wqk")
        nc.sync.dma_start(wqk_sb[:], wqk)
        wv_sb = const.tile([128, NCHUNK, H], F16, tag="wv")
        nc.sync.dma_start(wv_sb[:], wv)
        mask_sb = const.tile([128, 512], F16, tag="mask")
        nc.sync.dma_start(mask_sb[:], mask)

        # Persistent tiles.
        # v_aug slots: per batch [128, 256]: cols 0:64 = v s-half0, col 64 =
        # ones (denominator row), cols 65:128 = ZERO pad (so the outT
        # LDWEIGHTS is a 128-column load -> fast-weight-load kicks in),
        # cols 128:192 = v s-half1, col 192 = ones, 193:256 = zero pad.
        vaug = []
        for i in range(4):
            v_t = const.tile([128, GRP, 256], F16, tag=f"vaug{i}")
            nc.gpsimd.memset(v_t[:], 0.0)
            nc.gpsimd.memset(v_t[:, :, 64:65], 1.0)
            nc.gpsimd.memset(v_t[:, :, 192:193], 1.0)
            vaug.append(v_t)
        # q/k slots for the K=128 zero-padded weiT: rows 0:64 hold q (k),
        # rows 64:128 stay zero so the padded contraction adds nothing.
        qzs, kzs = [], []
        for i in range(3):
            q_t = const.tile([128, 1024], F16, tag=f"qz{i}")
            nc.gpsimd.memset(q_t[64:128, :], 0.0)
            qzs.append(q_t)
            k_t = const.tile([128, 1024], F16, tag=f"kz{i}")
            nc.gpsimd.memset(k_t[64:128, :], 0.0)
            kzs.append(k_t)

        xt_pool = ctx.enter_context(tc.tile_pool(name="xt", bufs=4))
        p0_pool = ctx.enter_context(tc.tile_pool(name="p0", bufs=16))
        o_pool = ctx.enter_context(tc.tile_pool(name="o", bufs=3))
        psqk_pool = ctx.enter_context(tc.tile_pool(name="psqk", bufs=2, space="PSUM"))
        psv_pool = ctx.enter_context(tc.tile_pool(name="psv", bufs=2, space="PSUM"))
        psw_pool = ctx.enter_context(tc.tile_pool(name="psw", bufs=2, space="PSUM"))
        pso_pool = ctx.enter_context(tc.tile_pool(name="pso", bufs=2, space="PSUM"))

        # Per-group state carried between pipeline stages.
        st = {}

        for i in range(ng + 2):
            g0 = i  # stage 0: input DMA
            g1 = i - 1  # stage 1: qk/v matmuls, evacuations, weiT, softmax
            g3 = i - 3  # stage 3: outT, output evac + DMA (P is 2 iters old)

            if 0 <= g1 < ng:
                s1 = st[g1]
                xt_sb = s1["xt"]
                qz, kz = qzs[g1 % 3], kzs[g1 % 3]
                s1["qz"], s1["kz"] = qz, kz
                # qk: per pair j, 3 accumulating MMs N=512 -> [q;k]
                # stacked. Evacuate q to rows 0:64 of the qz slot (cols
                # j*512+), k into kz; rows 64:128 are persistent zeros.
                for j in range(2):
                    psqk = psqk_pool.tile([128, 512], F32, tag="psqk")
                    for c in range(NCHUNK):
                        nc.tensor.matmul(
                            psqk[:],
                            lhsT=wqk_sb[:, c, :],
                            rhs=xt_sb[:, c, j * 512 : (j + 1) * 512],
                            start=(c == 0),
                            stop=(c == NCHUNK - 1),
                        )
                    nc.scalar.copy(
                        qz[0:64, j * 512 : (j + 1) * 512], psqk[0:64, :]
                    )
                    nc.vector.tensor_copy(
                        kz[0:64, j * 512 : (j + 1) * 512], psqk[64:128, :]
                    )

                # v direct in [t, h] layout: stationary = xt chunk t-half,
                # moving = Wv chunk. 8 regions x 3 accumulating MMs, N=64.
                psv = psv_pool.tile([128, 512], F32, tag="psv")
                for b in range(GRP):
                    for th in range(2):
                        off = b * 128 + th * 64
                        toff = b * T + th * 128
                        for c in range(NCHUNK):
                            nc.tensor.matmul(
                                psv[:, off : off + 64],
                                lhsT=xt_sb[:, c, toff : toff + 128],
                                rhs=wv_sb[:, c, :],
                                start=(c == 0),
                                stop=(c == NCHUNK - 1),
                            )
                v4 = vaug[g1 % 4]
                dst = v4.rearrange("p b (two g) -> p b two g", two=2)[
                    :, :, :, 0:64
                ]
                vsrc = psv[:].rearrange("p (b two f) -> p b two f", b=GRP, two=2)
                nc.vector.tensor_copy(dst, vsrc)
                s1["v4"] = v4

            g3_list = [g3] if 0 <= g3 else []
            if i == ng + 1:
                g3_list.append(ng - 1)  # drain: fold the last group in
            for g3 in g3_list:
                s3 = st[g3]
                v4 = s3["v4"]
                # outT: per batch, 2 accumulating MMs (s-halves). lhsT is the
                # 128-col padded v_aug slice: M=128 (rows 65:128 of the
                # output are zeros), col 64 = ones -> row 64 = softmax denom.
                # P is two iterations old: the exp->mask chain never stalls
                # these matmuls; they also fill this iteration's evac waits.
                pso = [
                    pso_pool.tile([128, 512], F32, tag="pso", name="pso")
                    for _ in range(2)
                ]
                for b in range(GRP):
                    j, bb = divmod(b, 2)
                    nc.tensor.matmul(
                        pso[j][:, bb * T : (bb + 1) * T],
                        lhsT=v4[:, b, 0:128],
                        rhs=s3["p0"][b][:, 128:384],
                        start=True,
                        stop=False,
                    )
                    # s-half1 contributes only to t >= 128 (causal).
                    nc.tensor.matmul(
                        pso[j][:, bb * T + 128 : (bb + 1) * T],
                        lhsT=v4[:, b, 128:256],
                        rhs=s3["p0"][b][:, 0:128],
                        start=False,
                        stop=True,
                    )

                o_sb = o_pool.tile([H + 1, GRP * T], F16, tag="o")
                nc.vector.tensor_copy(o_sb[:, 0 : 2 * T], pso[0][0 : H + 1, :])
                nc.vector.tensor_copy(o_sb[:, 2 * T : 4 * T], pso[1][0 : H + 1, :])
                nc.sync.dma_start(outT[g3], o_sb[:])
                del st[g3]

            if 0 <= g1 < ng:
                s1 = st[g1]
                s1["psw"], s1["p0"] = {}, {}
                for b in range(GRP):
                    _weiT(nc, psw_pool, s1, s1["qz"], s1["kz"], b)
                for b in range(GRP):
                    _softmax(nc, p0_pool, s1, mask_sb, b)

            if g0 < ng:
                xt_sb = xt_pool.tile([128, NCHUNK, GRP * T], F16, tag="xt")
                if i == 0:
                    # Prologue: split the first DMA so the very first qk
                    # matmuls (which read cols 0:512) start half a transfer
                    # earlier.
                    nc.sync.dma_start(xt_sb[:, :, 0:512], xt[g0][:, :, 0:512])
                    nc.sync.dma_start(
                        xt_sb[:, :, 512:1024], xt[g0][:, :, 512:1024]
                    )
                else:
                    nc.sync.dma_start(xt_sb[:], xt[g0])
                st[g0] = {"xt": xt_sb}


    nc.finalize()
    return nc


def _host_inputs(x, Wq, Wk, Wv):
    B_, T_, C_ = x.shape
    assert (B_, T_, C_) == (B, T, C), (B_, T_, C_)
    # xh[g, p, c, 256*i + t] = x[4g+i, t, 128c+p]
    xh = np.ascontiguousarray(
        x.reshape(B // GRP, GRP, T, NCHUNK, 128)
        .transpose(0, 4, 3, 1, 2)
        .reshape(B // GRP, 128, NCHUNK, GRP * T)
        .astype(np.float16)
    )
    wqk_h = np.ascontiguousarray(
        np.concatenate([Wq, Wk], axis=1).reshape(NCHUNK, 128, 128).transpose(1, 0, 2),
        dtype=np.float16,
    )
    wv_h = np.ascontiguousarray(
        Wv.reshape(NCHUNK, 128, H).transpose(1, 0, 2), dtype=np.float16
    )
    tri = np.triu(np.ones((128, 128), dtype=np.float16))
    mask_h = np.ascontiguousarray(np.concatenate([tri, tri, tri, tri], axis=1))
    return xh, wqk_h, wv_h, mask_h


def _gather(results):
    """Concatenate per-core outT, normalize, and restore [B, T, H] fp32."""
    outT = np.concatenate(
        [results[i]["outT"] for i in range(N_CORES)], axis=0
    ).astype(np.float32)  # [B/GRP, 65, GRP*T]
    outT = outT.reshape(B // GRP, H + 1, GRP, T)
    out = outT[:, 0:H] / outT[:, H : H + 1]  # [B/GRP, H, GRP, T]
    return np.ascontiguousarray(
        out.transpose(0, 2, 3, 1).reshape(B, T, H).astype(np.float32)
    )


def kernel(x, Wq, Wk, Wv):
    x = np.asarray(x, dtype=np.float32)
    Wq = np.asarray(Wq, dtype=np.float32)
    Wk = np.asarray(Wk, dtype=np.float32)
    Wv = np.asarray(Wv, dtype=np.float32)

    xh, wqk_h, wv_h, mask_h = _host_inputs(x, Wq, Wk, Wv)

    nc = build_nc(NG)
    in_maps = [
        {
            "xt": xh[i * NG : (i + 1) * NG],
            "wqk": wqk_h,
            "wv": wv_h,
            "mask": mask_h,
        }
        for i in range(N_CORES)
    ]
    res = run_bass_kernel_spmd(nc, in_maps, list(range(N_CORES)))
    return _gather(res.results)
